# revision 1
# baseline (speedup 1.0000x reference)
"""BlobSplatter Trainium2 kernel.

Per core (batch slice of 32), the splat exponent for blob n, batch b is a
quadratic polynomial in (cr, cc) = pixel-center coords:

  E[r,c] = mA*(cr-y)^2 + mC*(cc-x)^2 + nB*(cr-y)*(cc-x)
         = cr^2 * R0[c] + cr * R1[c] + 1 * R2[c]
  R0[c] = mA
  R1[c] = nB*cc - (2*mA*y + nB*x)
  R2[c] = mC*cc^2 - (2*mC*x + nB*y)*cc + (mA*y^2 + mC*x^2 + nB*x*y)

so E = VR.T @ R with a CONSTANT lhsT VR (rows cr^2, cr, 1) and a per-(b,n)
rhs R [3, 256] built densely on the DVE and interleaved into [3, *] tiles
by DMA.  The blob blend img = img*cur + cur unrolls to the Horner chain
acc_n = (acc_{n-1} + 1) * exp(E_n): one ACT exp + one DVE STT per blob.

Main-loop unit = (row-block m, group of 8 batches): PSUM E tile
[128, 2048] (free = b_local*256 + c), fp16 exp/blend tiles, f32 output.
"""

import sys

sys.path.insert(0, "/opt/trn_rl_repo")

import math
from contextlib import ExitStack

import numpy as np

import concourse.bacc as bacc
import concourse.mybir as mybir
from concourse import tile
from concourse.bass_utils import run_bass_kernel_spmd

N_CORES = 8
B_FULL = 256
BC = B_FULL // N_CORES  # 32 batches per core
T = 256
N_BLOBS = 8
H = 64
EPS = 1e-6

SIDE_RIGHT = np.array([1, 0, 1, 0, 1, 0, 1, 0], dtype=bool)
START_Y = np.array([0.1, 0.2, 0.3, 0.4, 0.5, 0.6, 0.7, 0.8], dtype=np.float32)
START_X = np.array([0.8, 0.7, 0.6, 0.5, 0.4, 0.3, 0.2, 0.1], dtype=np.float32)
START_S = 0.05
A_MIN = 0.5
A_MAX = 2.0

F32 = mybir.dt.float32
F16 = mybir.dt.float16
BF16 = mybir.dt.bfloat16
AF = mybir.ActivationFunctionType
ALU = mybir.AluOpType

BLEND_DT = F16

_CACHE = {}


def _bf16r(x):
    """round-to-nearest-even to bfloat16, returned as float32"""
    v = np.asarray(x, np.float32).view(np.uint32)
    r = (v + 0x7FFF + ((v >> 16) & 1)) & 0xFFFF0000
    return r.view(np.float32)


def _build_nc():
    nc = bacc.Bacc("TRN2", target_bir_lowering=False, debug=False, num_devices=N_CORES)

    positions = nc.dram_tensor("positions", [BC, 6], F32, kind="ExternalInput")
    W1 = nc.dram_tensor("W1", [N_BLOBS, 3, H], F32, kind="ExternalInput")
    b1 = nc.dram_tensor("b1", [N_BLOBS, H], F32, kind="ExternalInput")
    W2 = nc.dram_tensor("W2", [N_BLOBS, H, H], F32, kind="ExternalInput")
    b2 = nc.dram_tensor("b2", [N_BLOBS, H], F32, kind="ExternalInput")
    W3 = nc.dram_tensor("W3", [N_BLOBS, H, 5], F32, kind="ExternalInput")
    b3 = nc.dram_tensor("b3", [N_BLOBS, 5], F32, kind="ExternalInput")
    bsf = nc.dram_tensor("bsf", [1, 1], F32, kind="ExternalInput")
    out = nc.dram_tensor("out", [BC, T, T], F32, kind="ExternalOutput")
    dbg = None

    cc = ((np.arange(T) + 0.5) / T).astype(np.float32)
    c2 = (cc.astype(np.float64) ** 2)
    c2h = _bf16r(c2); c2m = _bf16r(c2 - c2h); c2l = _bf16r(c2 - c2h - c2m)
    crh = _bf16r(cc.astype(np.float64)); crl = _bf16r(cc - crh)
    one = np.ones(T, np.float32)
    # pairing with rhs rows [R0h,R0m,R0h,R0m,R0h,R0l, R1h,R1m,R1h,R1m,R1l, R2h,R2m,R2l]
    l14_np = np.stack([c2h, c2h, c2m, c2m, c2l, c2h, crh, crh, crl, crl, crh, one, one, one])
    import ml_dtypes
    L14 = nc.inline_tensor(np.ascontiguousarray(l14_np.astype(ml_dtypes.bfloat16)), "L14")
    ccB = nc.inline_tensor(np.ascontiguousarray(np.broadcast_to(cc, (128, T))), "ccB")
    cc2B = nc.inline_tensor(
        np.ascontiguousarray(np.broadcast_to((cc * cc).astype(np.float32), (128, T))),
        "cc2B",
    )
    # dense per-(b,n) start offsets: partition nb = 8*b + n
    nbl = np.arange(B_FULL) % N_BLOBS
    syx_np = np.stack([START_Y[nbl], START_X[nbl]], axis=1).astype(np.float32)
    SYX = nc.inline_tensor(np.ascontiguousarray(syx_np), "SYX")  # [256, 2]

    with tile.TileContext(nc) as tc:
        _body(nc, tc, positions, W1, b1, W2, b2, W3, b3, bsf, out, L14, ccB, cc2B, SYX, dbg)
    nc.compile()
    return nc


def _body(nc, tc, positions, W1, b1, W2, b2, W3, b3, bsf, out, L14, ccB, cc2B, SYX, dbg=None):
    with ExitStack() as ctx:
        cp = ctx.enter_context(tc.tile_pool(name="cp", bufs=1))

        # -------- constants / weights to SBUF --------
        l14t = cp.tile([14, T], BF16)
        nc.sync.dma_start(l14t[:], L14[:])
        ccb = cp.tile([128, T], F32)
        nc.gpsimd.dma_start(ccb[:], ccB[:])
        cc2b = cp.tile([128, T], F32)
        nc.gpsimd.dma_start(cc2b[:], cc2B[:])

        posR = cp.tile([3, BC], F32)
        nc.sync.dma_start(posR[:], positions[:].rearrange("b c -> c b")[0:3])
        posL = cp.tile([3, BC], F32)
        nc.sync.dma_start(posL[:], positions[:].rearrange("b c -> c b")[3:6])

        W1s = cp.tile([3, N_BLOBS * H], F32)
        nc.sync.dma_start(
            W1s[:].rearrange("i (n h) -> i n h", n=N_BLOBS),
            W1[:].rearrange("n i h -> i n h"),
        )
        # fold the reference's pos*100 into W1
        nc.vector.tensor_scalar_mul(W1s[:], W1s[:], 100.0)
        W2s = cp.tile([H, N_BLOBS * H], F32)
        nc.gpsimd.dma_start(
            W2s[:].rearrange("h (n k) -> h n k", n=N_BLOBS),
            W2[:].rearrange("n h k -> h n k"),
        )
        W3s = cp.tile([H, N_BLOBS * 5], F32)
        nc.sync.dma_start(
            W3s[:].rearrange("h (n k) -> h n k", n=N_BLOBS),
            W3[:].rearrange("n h k -> h n k"),
        )
        b1T = cp.tile([H, N_BLOBS], F32)
        nc.gpsimd.dma_start(b1T[:], b1[:].rearrange("n k -> k n"))
        b2T = cp.tile([H, N_BLOBS], F32)
        nc.sync.dma_start(b2T[:], b2[:].rearrange("n k -> k n"))
        b3T = cp.tile([5, N_BLOBS], F32)
        nc.gpsimd.dma_start(b3T[:], b3[:].rearrange("n k -> k n"))
        bsfB = cp.tile([128, 1], F32)
        nc.sync.dma_start(bsfB[:], bsf[:].broadcast_to((128, 1)))
        syxd = []
        for q in range(2):
            t_ = cp.tile([128, 2], F32, tag=f"syxd{q}", name="syxd")
            nc.sync.dma_start(t_[:], SYX[128 * q : 128 * q + 128, :])
            syxd.append(t_)

        mpihalf = cp.tile([128, 1], F32)
        nc.vector.memset(mpihalf[:], -math.pi / 2)

        psum = ctx.enter_context(tc.tile_pool(name="psum", bufs=2, space="PSUM"))

        # -------- encode MLP (feature-on-partition) --------
        bd_all = cp.tile([5, BC * N_BLOBS], F32)  # col = n*32 + b
        for n in range(N_BLOBS):
            pos = posR if SIDE_RIGHT[n] else posL
            mm = psum.tile([128, 2048], F32, tag="E", name="mm")
            nc.tensor.matmul(
                mm[:H, 0:BC], W1s[:, n * H : (n + 1) * H], pos[:], start=True, stop=True
            )
            h1 = cp.tile([H, BC], F32, tag="h1", bufs=2, name="h1")
            nc.vector.tensor_scalar(
                h1[:], mm[:H, 0:BC], b1T[:, n : n + 1], 0.0, ALU.add, ALU.max
            )
            mm2 = psum.tile([128, 2048], F32, tag="E", name="mm2")
            nc.tensor.matmul(
                mm2[:H, 0:BC], W2s[:, n * H : (n + 1) * H], h1[:], start=True, stop=True
            )
            h2 = cp.tile([H, BC], F32, tag="h2", bufs=2, name="h2")
            nc.vector.tensor_scalar(
                h2[:], mm2[:H, 0:BC], b2T[:, n : n + 1], 0.0, ALU.add, ALU.max
            )
            mm3 = psum.tile([128, 2048], F32, tag="E", name="mm3")
            nc.tensor.matmul(
                mm3[:5, 0:BC], W3s[:, n * 5 : (n + 1) * 5], h2[:], start=True, stop=True
            )
            nc.vector.tensor_scalar_add(
                bd_all[:].rearrange("p (b n) -> p n b", n=N_BLOBS)[:, n, :],
                mm3[:5, 0:BC],
                b3T[:, n : n + 1],
            )

        # -------- params, dense layout: partition nb = 8*b + n --------
        RD = []  # per q: [128, 768] rows R0|R1|R2
        BDDBG = []; WKDBG = []; YXDBG = []
        for q in range(2):
            bdd = cp.tile([128, 5], F32, tag=f"bdd{q}", name="bdd")
            for i in range(5):
                eng = nc.gpsimd if i % 2 else nc.sync
                eng.dma_start(
                    bdd[:, i : i + 1],
                    bd_all[i : i + 1, 128 * q : 128 * q + 128],
                )
            wk = cp.tile([128, 24], F32, tag=f"wk{q}", name="wk")

            def col(i):
                return wk[:, i : i + 1]

            sg = cp.tile([128, 4], F32, tag=f"sg{q}", name="sg")
            nc.scalar.activation(sg[:, 0:2], bdd[:, 0:2], AF.Sigmoid)
            nc.scalar.activation(sg[:, 2:4], bdd[:, 3:5], AF.Sigmoid)
            yx = cp.tile([128, 2], F32, tag=f"yx{q}", name="yx")
            nc.vector.tensor_add(yx[:], sg[:, 0:2], syxd[q][:])
            y_, x_ = yx[:, 0:1], yx[:, 1:2]
            s_ = col(0)
            nc.vector.tensor_scalar(s_, bdd[:, 2:3], START_S, bsfB[:, 0:1], ALU.add, ALU.mult)
            a_ = col(1)
            nc.vector.tensor_scalar(a_, sg[:, 2:3], A_MAX - A_MIN, A_MIN, ALU.mult, ALU.add)
            c_ = col(2)
            # cos(th) = -sin(th - pi/2); th - pi/2 stays inside Sin's domain
            nc.scalar.activation(c_, sg[:, 3:4], AF.Sin, bias=mpihalf[:, 0:1], scale=math.pi)
            nc.vector.tensor_scalar_mul(c_, c_, -1.0)
            sn_ = col(3)
            nc.scalar.activation(sn_, sg[:, 3:4], AF.Sin, bias=0.0, scale=math.pi)

            sa = col(4)
            nc.vector.tensor_mul(sa, s_, a_)
            nc.vector.tensor_scalar_add(sa, sa, EPS)
            ia2 = col(5)
            nc.vector.reciprocal(ia2, sa)
            nc.vector.tensor_mul(ia2, ia2, ia2)
            ib2 = col(6)
            nc.vector.tensor_scalar_add(ib2, a_, EPS)
            nc.vector.reciprocal(ib2, ib2)
            nc.vector.tensor_mul(ib2, ib2, s_)
            nc.vector.tensor_scalar_add(ib2, ib2, EPS)
            nc.vector.reciprocal(ib2, ib2)
            nc.vector.tensor_mul(ib2, ib2, ib2)
            c2 = col(7)
            nc.vector.tensor_mul(c2, c_, c_)
            sn2 = col(8)
            nc.vector.tensor_mul(sn2, sn_, sn_)
            csn = col(9)
            nc.vector.tensor_mul(csn, c_, sn_)
            t1 = col(10)
            nc.vector.tensor_mul(t1, c2, ia2)
            t2 = col(11)
            nc.vector.tensor_mul(t2, sn2, ib2)
            mA = col(12)
            nc.vector.tensor_add(mA, t1, t2)
            nc.vector.tensor_scalar_mul(mA, mA, -0.5)
            t3 = col(13)
            nc.vector.tensor_mul(t3, sn2, ia2)
            t4 = col(14)
            nc.vector.tensor_mul(t4, c2, ib2)
            mC = col(15)
            nc.vector.tensor_add(mC, t3, t4)
            nc.vector.tensor_scalar_mul(mC, mC, -0.5)
            dd = col(16)
            nc.vector.tensor_sub(dd, ia2, ib2)
            nB = col(17)
            nc.vector.scalar_tensor_tensor(nB, csn, -1.0, dd, ALU.mult, ALU.mult)

            # Vandermonde coeffs
            al = col(18)  # 2*mA*y + nB*x
            nc.vector.scalar_tensor_tensor(al, mA, 2.0, y_, ALU.mult, ALU.mult)
            u2 = col(19)
            nc.vector.tensor_mul(u2, nB, x_)
            nc.vector.tensor_add(al, al, u2)
            mbe = col(20)  # -(2*mC*x + nB*y)
            nc.vector.scalar_tensor_tensor(mbe, mC, -2.0, x_, ALU.mult, ALU.mult)
            u3 = col(21)
            nc.vector.tensor_mul(u3, nB, y_)
            nc.vector.tensor_sub(mbe, mbe, u3)
            ga = col(22)  # mA*y^2 + mC*x^2 + nB*x*y
            y2 = col(23)
            nc.vector.tensor_mul(y2, y_, y_)
            nc.vector.tensor_mul(ga, mA, y2)
            x2 = col(23)
            nc.vector.tensor_mul(x2, x_, x_)
            u4 = col(19)
            nc.vector.tensor_mul(u4, mC, x2)
            nc.vector.tensor_add(ga, ga, u4)
            xy = col(23)
            nc.vector.tensor_mul(xy, x_, y_)
            u5 = col(19)
            nc.vector.tensor_mul(u5, nB, xy)
            nc.vector.tensor_add(ga, ga, u5)

            # dense R rows [128, 768]: R0 | R1 | R2
            rd = cp.tile([128, 3 * T], F32, tag=f"rd{q}", name="rd")
            nc.vector.tensor_scalar(rd[:, 0:T], ccb[:], 0.0, mA, ALU.mult, ALU.add)
            nc.vector.tensor_scalar(rd[:, T : 2 * T], ccb[:], nB, al, ALU.mult, ALU.subtract)
            nc.vector.tensor_scalar(rd[:, 2 * T : 3 * T], cc2b[:], mC, ga, ALU.mult, ALU.add)
            nc.vector.scalar_tensor_tensor(
                rd[:, 2 * T : 3 * T], ccb[:], mbe, rd[:, 2 * T : 3 * T], ALU.mult, ALU.add
            )
            rdh = cp.tile([128, 3 * T], BF16, tag=f"rdh{q}", name="rdh")
            nc.vector.tensor_copy(rdh[:], rd[:])
            rem = cp.tile([128, 3 * T], F32, tag=f"rem{q}", name="rem")
            nc.vector.tensor_sub(rem[:], rd[:], rdh[:])
            rdm = cp.tile([128, 3 * T], BF16, tag=f"rdm{q}", name="rdm")
            nc.vector.tensor_copy(rdm[:], rem[:])
            rdl = cp.tile([128, 3 * T], BF16, tag=f"rdl{q}", name="rdl")
            nc.vector.tensor_sub(rdl[:], rem[:], rdm[:])
            RD.append((rdh, rdm, rdl))
            BDDBG.append(bdd); WKDBG.append(wk); YXDBG.append(yx)

        # -------- RI fill: one big [14, 256*256] bf16 tile --------
        # rhs rows: 0:R0h 1:R0m 2:R0h 3:R0m 4:R0h 5:R0l 6:R1h 7:R1m 8:R1h
        #           9:R1m 10:R1l 11:R2h 12:R2m 13:R2l ; free = nb*256 + c
        ri = cp.tile([14, B_FULL * T], BF16)
        ROW_SRC = [
            (0, 0), (1, 0), (0, 0), (1, 0), (0, 0), (2, 0),
            (0, 1), (1, 1), (0, 1), (1, 1), (2, 1),
            (0, 2), (1, 2), (2, 2),
        ]
        for q in range(2):
            splits = RD[q]
            for row, (which, colr) in enumerate(ROW_SRC):
                eng = nc.sync if row < 9 else nc.gpsimd
                eng.dma_start(
                    ri[row : row + 1, q * 128 * T : (q + 1) * 128 * T],
                    splits[which][:, colr * T : (colr + 1) * T],
                )

        # -------- main loop: suffix sums S_k in PSUM, out = sum_k exp(S_k) ----
        # The two row-block units (m=0,1) of each batch-group run in lockstep:
        # ACT exps one unit's PSUM while PE accumulates the other's.
        tp = ctx.enter_context(tc.tile_pool(name="tp", bufs=2))
        accp = ctx.enter_context(tc.tile_pool(name="accp", bufs=3))
        outp = ctx.enter_context(tc.tile_pool(name="outp", bufs=2))
        riv = ri[:].rearrange("k (b n c) -> k n b c", b=BC, n=N_BLOBS)
        for bg in range(4):
            Es = [psum.tile([128, 2048], F32, tag="E", name=f"E{m}") for m in range(2)]
            acc = [None, None]
            for kb in reversed(range(N_BLOBS)):
                for m in range(2):
                    for bl2 in range(4):
                        b0 = 8 * bg + 2 * bl2
                        nc.tensor.matmul(
                            Es[m][:, 512 * bl2 : 512 * bl2 + 512],
                            l14t[:, 128 * m : 128 * m + 128],
                            riv[:, kb, b0 : b0 + 2, :],
                            start=(kb == N_BLOBS - 1),
                            stop=(kb == 0),
                            skip_group_check=True,
                        )
                for m in range(2):
                    if kb == N_BLOBS - 1:
                        a0 = accp.tile([128, 2048], BLEND_DT, tag="acc", name="a0")
                        nc.scalar.activation(a0[:], Es[m][:], AF.Exp)
                        acc[m] = a0
                    elif kb > 0:
                        t = tp.tile([128, 2048], BLEND_DT, tag="t", name="t")
                        nc.scalar.activation(t[:], Es[m][:], AF.Exp)
                        a2 = accp.tile([128, 2048], BLEND_DT, tag="acc", name="a2")
                        nc.vector.tensor_add(a2[:], acc[m][:], t[:])
                        acc[m] = a2
                    else:
                        t = tp.tile([128, 2048], BLEND_DT, tag="t", name="tl")
                        nc.scalar.activation(t[:], Es[m][:], AF.Exp)
                        of = outp.tile([128, 2048], F32, tag="of", name="of")
                        for hh in range(2):
                            sl = slice(1024 * hh, 1024 * hh + 1024)
                            nc.vector.tensor_add(of[:, sl], acc[m][:, sl], t[:, sl])
                            nc.sync.dma_start(
                                out[8 * bg + 4 * hh : 8 * bg + 4 * hh + 4,
                                    128 * m : 128 * m + 128, :]
                                .rearrange("b r c -> r b c"),
                                of[:, sl].rearrange("r (b c) -> r b c", c=T),
                            )


def _get_nc():
    if "nc" not in _CACHE:
        _CACHE["nc"] = _build_nc()
    return _CACHE["nc"]


def _make_in_maps(inputs):
    pos = np.asarray(inputs["positions"], dtype=np.float32)
    assert pos.shape == (B_FULL, 6)
    assert int(inputs["target_size"]) == T
    shared = {
        "W1": np.ascontiguousarray(np.asarray(inputs["W1"], np.float32)),
        "b1": np.ascontiguousarray(np.asarray(inputs["b1"], np.float32)),
        "W2": np.ascontiguousarray(np.asarray(inputs["W2"], np.float32)),
        "b2": np.ascontiguousarray(np.asarray(inputs["b2"], np.float32)),
        "W3": np.ascontiguousarray(np.asarray(inputs["W3"], np.float32)),
        "b3": np.ascontiguousarray(np.asarray(inputs["b3"], np.float32)),
        "bsf": np.asarray(inputs["blobs_scale_factor"], np.float32).reshape(1, 1),
    }
    return [
        {"positions": np.ascontiguousarray(pos[c * BC : (c + 1) * BC]), **shared}
        for c in range(N_CORES)
    ]


def run(trace=False, **inputs):
    nc = _get_nc()
    in_maps = _make_in_maps(inputs)
    res = run_bass_kernel_spmd(nc, in_maps, list(range(N_CORES)), trace=trace)
    outp = np.concatenate([r["out"] for r in res.results], axis=0)
    return outp, res


def kernel(**inputs):
    return run(**inputs)[0]

